# revision 5
# baseline (speedup 1.0000x reference)
"""MeshUnpool Trainium2 kernel.

For every fine edge slot s in [0, 16384):
  - if s is a kept slot (s == keep_idx[j] for some j): out[s] = x_coarse[j]
  - else: out[s] = x_coarse[argmin_j |keep_idx[j] - s|]  (first-min tie-break)

Every output row is a gathered x_coarse row; the device computes the gather
index per slot with an O(E_fine) scan algorithm instead of the naive
(16384 x 8192) distance matrix:

  1. scatter packed keys  key1 = slot*128 + j_hi, key2 = slot*64 + j_lo
     (j = j_hi*64 + j_lo) into a 16384-row DRAM table at the kept slots
     (indirect DMA); missing slots hold sentinels.
  2. prefix-max scan of key1/key2 over slots -> nearest kept slot <= s with
     its j riding along; suffix-min scan -> nearest kept slot >= s.
     Scans run per-partition with tensor_tensor_scan plus a transposed
     cross-partition carry fixup.
  3. elementwise distance compare + first-min tie-break -> src_idx per slot.
  4. a one-hot matmul extracts this core's 2048 indices, then 16 indirect
     row-gather DMAs (128 rows x 2 KB) pull the output rows from x_coarse.

Work is sharded over 8 cores by rows of the fine-edge dim; x_coarse and
keep_idx are replicated; each core fills its 2048-row slice.
"""

import os
import sys

import numpy as np

E_FINE = 16384
E_COARSE = 8192
C = 512
N_CORES = 8
SLICE = E_FINE // N_CORES  # 2048
P = 128
NBLK = SLICE // P  # 16 gather blocks per core
KC = E_COARSE // P  # 64 keep columns (j = 64*p + c)

L_SENT = -2097152.0  # -2^21
R_SENT = 4194304.0   # +2^22
BUMP = R_SENT - L_SENT

_NC_CACHE = {}
_DUMP = None  # debug hook: _DUMP(name, ap) dumps an AP to a DRAM tensor


def _dump(name, ap):
    if _DUMP is not None:
        _DUMP(name, ap)


def _ensure_paths():
    for p in ("/opt/trn_rl_repo", "/root/.axon_site/_ro/trn_rl_repo"):
        if os.path.isdir(p) and p not in sys.path:
            sys.path.append(p)


def build_program(nc, bass, mybir, tile):
    f32 = mybir.dt.float32
    i32 = mybir.dt.int32
    Alu = mybir.AluOpType

    xc = nc.dram_tensor("xc", [E_COARSE, C], f32, kind="ExternalInput")
    keep = nc.dram_tensor("keep", [P, KC], i32, kind="ExternalInput")
    jlo_in = nc.dram_tensor("jlo", [P, KC], f32, kind="ExternalInput")
    parg = nc.dram_tensor("parg", [P, 1], f32, kind="ExternalInput")
    ident = nc.dram_tensor("ident", [P, P], f32, kind="ExternalInput")
    sel = nc.dram_tensor("sel", [P, NBLK], f32, kind="ExternalInput")
    y = nc.dram_tensor("y", [SLICE, C], f32, kind="ExternalOutput")

    with tile.TileContext(nc) as tc:
        with (
            tc.tile_pool(name="sb", bufs=1) as sb,
            tc.tile_pool(name="ps", bufs=1, space="PSUM") as ps,
            tc.tile_pool(name="dr", bufs=1, space="DRAM") as dr,
            tc.tile_pool(name="gp", bufs=4) as gp,
        ):
            scratch = dr.tile([E_FINE, 2], f32)

            keep_t = sb.tile([P, KC], i32)
            nc.sync.dma_start(keep_t[:], keep[:])
            jlo_t = sb.tile([P, KC], f32)
            nc.sync.dma_start(jlo_t[:], jlo_in[:])
            parg_t = sb.tile([P, 1], f32)
            nc.sync.dma_start(parg_t[:], parg[:])
            ident_t = sb.tile([P, P], f32)
            nc.sync.dma_start(ident_t[:], ident[:])
            sel_t = sb.tile([P, NBLK], f32)
            nc.sync.dma_start(sel_t[:], sel[:])

            # sentinel-fill the key table, then scatter the kept keys over it
            sent_t = sb.tile([P, 2 * P], f32)
            nc.vector.memset(sent_t[:], L_SENT)
            nc.sync.dma_start(
                scratch[:].rearrange("(p k) c -> p (k c)", p=P), sent_t[:]
            )

            keep_f = sb.tile([P, KC], f32)
            nc.vector.tensor_copy(keep_f[:], keep_t[:])
            kv = sb.tile([P, KC, 2], f32)
            nc.vector.tensor_scalar(
                kv[:, :, 0], keep_f[:], 128.0, parg_t[:, 0:1], Alu.mult, Alu.add
            )
            nc.vector.scalar_tensor_tensor(
                kv[:, :, 1], keep_f[:], 64.0, jlo_t[:], Alu.mult, Alu.add
            )
            for c in range(KC):
                nc.gpsimd.indirect_dma_start(
                    out=scratch[:],
                    out_offset=bass.IndirectOffsetOnAxis(
                        ap=keep_t[:, c : c + 1], axis=0
                    ),
                    in_=kv[:, c, :],
                    in_offset=None,
                )

            # p-major readback: partition p holds slots [128p, 128p+128)
            rb = sb.tile([P, P, 2], f32)
            nc.sync.dma_start(rb[:], scratch[:].rearrange("(p k) c -> p k c", p=P))
            key1 = sb.tile([P, P], f32)
            nc.vector.tensor_copy(key1[:], rb[:, :, 0])
            key2 = sb.tile([P, P], f32)
            nc.vector.tensor_copy(key2[:], rb[:, :, 1])
            _dump("d_key1", key1[:])
            _dump("d_key2", key2[:])

            # right-scan keys: sentinels flipped to +R_SENT
            msk = sb.tile([P, P], f32)
            nc.vector.tensor_scalar(msk[:], key1[:], L_SENT, None, Alu.is_equal)
            r1 = sb.tile([P, P], f32)
            nc.vector.scalar_tensor_tensor(
                r1[:], msk[:], BUMP, key1[:], Alu.mult, Alu.add
            )
            r2 = sb.tile([P, P], f32)
            nc.vector.scalar_tensor_tensor(
                r2[:], msk[:], BUMP, key2[:], Alu.mult, Alu.add
            )

            # per-partition scans (free axis); suffix scans via reversed APs
            l1s = sb.tile([P, P], f32)
            nc.vector.tensor_tensor_scan(
                l1s[:], key1[:], key1[:], L_SENT, Alu.max, Alu.max
            )
            l2s = sb.tile([P, P], f32)
            nc.vector.tensor_tensor_scan(
                l2s[:], key2[:], key2[:], L_SENT, Alu.max, Alu.max
            )
            r1s = sb.tile([P, P], f32)
            nc.vector.tensor_tensor_scan(
                r1s[:, P - 1 :: -1],
                r1[:, P - 1 :: -1],
                r1[:, P - 1 :: -1],
                R_SENT,
                Alu.min,
                Alu.min,
            )
            r2s = sb.tile([P, P], f32)
            nc.vector.tensor_tensor_scan(
                r2s[:, P - 1 :: -1],
                r2[:, P - 1 :: -1],
                r2[:, P - 1 :: -1],
                R_SENT,
                Alu.min,
                Alu.min,
            )

            # cross-partition carry: transpose per-partition totals, exclusive
            # scan along the row, transpose back, combine
            totL = sb.tile([P, 2], f32)
            nc.vector.tensor_copy(totL[:, 0:1], l1s[:, P - 1 : P])
            nc.vector.tensor_copy(totL[:, 1:2], l2s[:, P - 1 : P])
            totR = sb.tile([P, 2], f32)
            nc.vector.tensor_copy(totR[:, 0:1], r1s[:, 0:1])
            nc.vector.tensor_copy(totR[:, 1:2], r2s[:, 0:1])
            totL_tp = ps.tile([2, P], f32)
            nc.tensor.transpose(totL_tp[:], totL[:], ident_t[:])
            totL_T = sb.tile([2, P], f32)
            nc.vector.tensor_copy(totL_T[:], totL_tp[:])
            totR_tp = ps.tile([2, P], f32)
            nc.tensor.transpose(totR_tp[:], totR[:], ident_t[:])
            totR_T = sb.tile([2, P], f32)
            nc.vector.tensor_copy(totR_T[:], totR_tp[:])

            exL = sb.tile([2, P], f32)
            nc.vector.memset(exL[:, 0:1], L_SENT)
            nc.vector.tensor_tensor_scan(
                exL[:, 1:P],
                totL_T[:, 0 : P - 1],
                totL_T[:, 0 : P - 1],
                L_SENT,
                Alu.max,
                Alu.max,
            )
            exR = sb.tile([2, P], f32)
            nc.vector.memset(exR[:, P - 1 : P], R_SENT)
            nc.vector.tensor_tensor_scan(
                exR[:, P - 2 :: -1],
                totR_T[:, P - 1 : 0 : -1],
                totR_T[:, P - 1 : 0 : -1],
                R_SENT,
                Alu.min,
                Alu.min,
            )
            exL_tp = ps.tile([P, 2], f32)
            nc.tensor.transpose(exL_tp[:], exL[:], ident_t[0:2, 0:2])
            carryL = sb.tile([P, 2], f32)
            nc.vector.tensor_copy(carryL[:], exL_tp[:])
            exR_tp = ps.tile([P, 2], f32)
            nc.tensor.transpose(exR_tp[:], exR[:], ident_t[0:2, 0:2])
            carryR = sb.tile([P, 2], f32)
            nc.vector.tensor_copy(carryR[:], exR_tp[:])
            nc.vector.tensor_scalar_max(l1s[:], l1s[:], carryL[:, 0:1])
            nc.vector.tensor_scalar_max(l2s[:], l2s[:], carryL[:, 1:2])
            nc.vector.tensor_scalar_min(r1s[:], r1s[:], carryR[:, 0:1])
            nc.vector.tensor_scalar_min(r2s[:], r2s[:], carryR[:, 1:2])
            _dump("d_l1s", l1s[:])
            _dump("d_l2s", l2s[:])
            _dump("d_r1s", r1s[:])
            _dump("d_r2s", r2s[:])

            # decode: slot = key1>>7, j = ((key1&127)<<6) | (key2&63)
            l1i = sb.tile([P, P], i32)
            nc.vector.tensor_copy(l1i[:], l1s[:])
            l2i = sb.tile([P, P], i32)
            nc.vector.tensor_copy(l2i[:], l2s[:])
            r1i = sb.tile([P, P], i32)
            nc.vector.tensor_copy(r1i[:], r1s[:])
            r2i = sb.tile([P, P], i32)
            nc.vector.tensor_copy(r2i[:], r2s[:])

            slot_l = sb.tile([P, P], i32)
            nc.vector.tensor_scalar(slot_l[:], l1i[:], 7, None, Alu.arith_shift_right)
            slot_r = sb.tile([P, P], i32)
            nc.vector.tensor_scalar(slot_r[:], r1i[:], 7, None, Alu.arith_shift_right)
            jhl = sb.tile([P, P], i32)
            nc.vector.tensor_scalar(
                jhl[:], l1i[:], 127, 6, Alu.bitwise_and, Alu.arith_shift_left
            )
            jll = sb.tile([P, P], i32)
            nc.vector.tensor_scalar(jll[:], l2i[:], 63, None, Alu.bitwise_and)
            jl = sb.tile([P, P], i32)
            nc.vector.tensor_tensor(jl[:], jhl[:], jll[:], Alu.bitwise_or)
            jhr = sb.tile([P, P], i32)
            nc.vector.tensor_scalar(
                jhr[:], r1i[:], 127, 6, Alu.bitwise_and, Alu.arith_shift_left
            )
            jlr = sb.tile([P, P], i32)
            nc.vector.tensor_scalar(jlr[:], r2i[:], 63, None, Alu.bitwise_and)
            jr = sb.tile([P, P], i32)
            nc.vector.tensor_tensor(jr[:], jhr[:], jlr[:], Alu.bitwise_or)

            pos = sb.tile([P, P], i32)
            nc.gpsimd.iota(pos[:], pattern=[[1, P]], base=0, channel_multiplier=P)
            dl = sb.tile([P, P], i32)
            nc.vector.tensor_tensor(dl[:], pos[:], slot_l[:], Alu.subtract)
            drr = sb.tile([P, P], i32)
            nc.vector.tensor_tensor(drr[:], slot_r[:], pos[:], Alu.subtract)
            m_l = sb.tile([P, P], i32)
            nc.vector.tensor_tensor(m_l[:], dl[:], drr[:], Alu.is_lt)
            m_r = sb.tile([P, P], i32)
            nc.vector.tensor_tensor(m_r[:], drr[:], dl[:], Alu.is_lt)
            src = sb.tile([P, P], i32)
            nc.vector.tensor_tensor(src[:], jl[:], jr[:], Alu.min)
            nc.vector.copy_predicated(src[:], m_r[:], jr[:])
            nc.vector.copy_predicated(src[:], m_l[:], jl[:])
            _dump("d_src", src[:])
            _dump("d_pos", pos[:])
            srcf = sb.tile([P, P], f32)
            nc.vector.tensor_copy(srcf[:], src[:])

            # extract this core's 16 blocks of 128 indices: G[r, g] =
            # src[16m+g, r] via one-hot matmul, then gather + write out
            g_ps = ps.tile([P, NBLK], f32)
            nc.tensor.matmul(g_ps[:], srcf[:], sel_t[:], start=True, stop=True)
            g_i = sb.tile([P, NBLK], i32)
            nc.vector.tensor_copy(g_i[:], g_ps[:])
            _dump("d_gi", g_i[:])

            for b in range(NBLK):
                gt = gp.tile([P, C], f32, tag="g")
                nc.gpsimd.indirect_dma_start(
                    out=gt[:],
                    out_offset=None,
                    in_=xc[:],
                    in_offset=bass.IndirectOffsetOnAxis(
                        ap=g_i[:, b : b + 1], axis=0
                    ),
                )
                nc.sync.dma_start(y[b * P : (b + 1) * P, :], gt[:])

    return {"y": y}


def host_inputs(x_coarse, keep_idx):
    x_coarse = np.ascontiguousarray(np.asarray(x_coarse), dtype=np.float32)
    keep = (
        np.ascontiguousarray(np.asarray(keep_idx), dtype=np.int32).reshape(P, KC)
    )
    jlo = np.tile(np.arange(KC, dtype=np.float32), (P, 1))
    parg_a = np.arange(P, dtype=np.float32).reshape(P, 1)
    ident_a = np.eye(P, dtype=np.float32)
    base = {
        "xc": x_coarse,
        "keep": keep,
        "jlo": np.ascontiguousarray(jlo),
        "parg": parg_a,
        "ident": ident_a,
    }
    in_maps = []
    for m in range(N_CORES):
        sel_a = np.zeros((P, NBLK), dtype=np.float32)
        sel_a[16 * m + np.arange(NBLK), np.arange(NBLK)] = 1.0
        in_maps.append(dict(base, sel=sel_a))
    return in_maps


def _get_nc():
    if "nc" in _NC_CACHE:
        return _NC_CACHE["nc"]
    _ensure_paths()
    from concourse import bass, mybir
    import concourse.bacc as bacc
    import concourse.tile as tile

    nc = bacc.Bacc("TRN2", target_bir_lowering=False, debug=False)
    build_program(nc, bass, mybir, tile)
    nc.compile()
    _NC_CACHE["nc"] = nc
    return nc


def run_on_hw(in_maps, trace=False, **kwargs):
    _ensure_paths()
    from concourse.bass_utils import run_bass_kernel_spmd

    nc = _get_nc()
    return run_bass_kernel_spmd(
        nc, in_maps, core_ids=list(range(N_CORES)), trace=trace, **kwargs
    )


def kernel(x_coarse, keep_idx, E_fine=None, **_unused):
    in_maps = host_inputs(x_coarse, keep_idx)
    res = run_on_hw(in_maps)
    out = np.concatenate([res.results[m]["y"] for m in range(N_CORES)], axis=0)
    return np.ascontiguousarray(out.astype(np.float32, copy=False))


# revision 12
# speedup vs baseline: 1.4435x; 1.4435x over previous
"""MeshUnpool Trainium2 kernel.

For every fine edge slot s in [0, 16384):
  - if s is a kept slot (s == keep_idx[j] for some j): out[s] = x_coarse[j]
  - else: out[s] = x_coarse[argmin_j |keep_idx[j] - s|]  (first-min tie-break)

Every output row is a gathered x_coarse row; the device computes the gather
index per slot with an O(E_fine) scan algorithm instead of the naive
(16384 x 8192) distance matrix:

  1. scatter packed keys  key1 = slot*128 + j_hi, key2 = slot*64 + j_lo
     (j = j_hi*64 + j_lo) into a 16384-row DRAM table at the kept slots
     (indirect DMA); missing slots hold sentinels.
  2. prefix-max scan of key1/key2 over slots -> nearest kept slot <= s with
     its j riding along; suffix-min scan -> nearest kept slot >= s.
     Scans run per-partition with tensor_tensor_scan plus a transposed
     cross-partition carry fixup.
  3. elementwise distance compare + first-min tie-break -> src_idx per slot.
  4. a one-hot matmul extracts this core's 2048 indices, then 16 indirect
     row-gather DMAs (128 rows x 2 KB) pull the output rows from x_coarse.

Work is sharded over 8 cores by rows of the fine-edge dim; x_coarse and
keep_idx are replicated; each core fills its 2048-row slice.
"""

import os
import sys

import numpy as np

E_FINE = 16384
E_COARSE = 8192
C = 512
N_CORES = 8
SLICE = E_FINE // N_CORES  # 2048
P = 128
NBLK = SLICE // P  # 16 gather blocks per core
KC = E_COARSE // P  # 64 keep columns (j = 64*p + c)

KEY_OFF = 2097152.0  # +2^21 added to keys so "missing slot" == 0.0
R_SENT = 8388608.0   # +2^23: flipped sentinel for the suffix-min scans

_NC_CACHE = {}
_DUMP = None  # debug hook: _DUMP(name, ap) dumps an AP to a DRAM tensor


def _dump(name, ap):
    if _DUMP is not None:
        _DUMP(name, ap)


def _ensure_paths():
    for p in ("/opt/trn_rl_repo", "/root/.axon_site/_ro/trn_rl_repo"):
        if os.path.isdir(p) and p not in sys.path:
            sys.path.append(p)


def build_program(nc, bass, mybir, tile):
    f32 = mybir.dt.float32
    i32 = mybir.dt.int32
    Alu = mybir.AluOpType

    i16 = mybir.dt.int16

    xc = nc.dram_tensor("xc", [E_COARSE, C], f32, kind="ExternalInput")
    # keep_w[p, c] = keep_idx[c*128 + p]  (token order for dma_scatter_add)
    keep_w = nc.dram_tensor("keep_w", [P, KC], i32, kind="ExternalInput")
    # keep16[q, u] = keep_idx[u*16 + (q % 16)]  (wrapped-16 scatter indices)
    keep16 = nc.dram_tensor("keep16", [P, E_COARSE // 16], i16, kind="ExternalInput")
    # jhi[p, c] = j_hi(t) + 2^21,  jlo[p, 0] = j_lo(t) + 2^21  for t = c*128+p
    jhi_in = nc.dram_tensor("jhi", [P, KC], f32, kind="ExternalInput")
    jlo_in = nc.dram_tensor("jlo", [P, 1], f32, kind="ExternalInput")
    ident = nc.dram_tensor("ident", [P, P], f32, kind="ExternalInput")
    sel = nc.dram_tensor("sel", [P, NBLK], f32, kind="ExternalInput")
    y = nc.dram_tensor("y", [SLICE, C], f32, kind="ExternalOutput")

    with tile.TileContext(nc) as tc:
        with (
            tc.tile_pool(name="sb", bufs=1) as sb,
            tc.tile_pool(name="ps", bufs=1, space="PSUM") as ps,
            tc.tile_pool(name="dr", bufs=1, space="DRAM") as dr,
            tc.tile_pool(name="gp", bufs=4) as gp,
        ):
            # key table: rows 256 B apart (dma_scatter_add stride granule);
            # only the first 8 B (2 f32 keys) of each row are used
            table = dr.tile([E_FINE, 64], f32)
            t3 = table[:].rearrange("(p k) c -> p k c", p=P)[:, :, 0:2]

            keep_t = sb.tile([P, KC], i32)
            nc.sync.dma_start(keep_t[:], keep_w[:])
            keep16_t = sb.tile([P, E_COARSE // 16], i16)
            nc.sync.dma_start(keep16_t[:], keep16[:])
            jhi_t = sb.tile([P, KC], f32)
            nc.sync.dma_start(jhi_t[:], jhi_in[:])
            jlo_t = sb.tile([P, 1], f32)
            nc.sync.dma_start(jlo_t[:], jlo_in[:])
            ident_t = sb.tile([P, P], f32)
            nc.sync.dma_start(ident_t[:], ident[:])
            sel_t = sb.tile([P, NBLK], f32)
            nc.sync.dma_start(sel_t[:], sel[:])

            # zero-init the used 8 B of every table row, then scatter-add the
            # kept keys (key + 2^21) so missing slots read back as 0.0
            zer_t = sb.tile([P, 2 * P], f32)
            nc.vector.memset(zer_t[:], 0.0)
            nc.sync.dma_start(t3, zer_t[:].rearrange("p (k c) -> p k c", c=2))

            keep_f = sb.tile([P, KC], f32)
            nc.vector.tensor_copy(keep_f[:], keep_t[:])
            kv = sb.tile([P, KC, 2], f32)
            nc.vector.scalar_tensor_tensor(
                kv[:, :, 0], keep_f[:], 128.0, jhi_t[:], Alu.mult, Alu.add
            )
            nc.vector.tensor_scalar(
                kv[:, :, 1], keep_f[:], 64.0, jlo_t[:, 0:1], Alu.mult, Alu.add
            )
            # the SWDGE ring takes at most ~1024 descriptors per op: split
            # the 8192-token scatter into two 4096-token halves
            H = E_COARSE // 2
            for h in range(2):
                nc.gpsimd.dma_scatter_add(
                    out_ap=table[:, 0:2],
                    in_ap=kv[:, h * (KC // 2) : (h + 1) * (KC // 2), :],
                    idxs_ap=keep16_t[:, h * (H // 16) : (h + 1) * (H // 16)],
                    num_idxs=H,
                    num_idxs_reg=H,
                    elem_size=2,
                    elem_step=64,
                )

            # p-major readback: partition p holds slots [128p, 128p+128)
            rb = sb.tile([P, P, 2], f32)
            nc.sync.dma_start(rb[:], t3)
            key1 = sb.tile([P, P], f32)
            nc.vector.tensor_copy(key1[:], rb[:, :, 0])
            key2 = sb.tile([P, P], f32)
            nc.vector.tensor_copy(key2[:], rb[:, :, 1])
            _dump("d_key1", key1[:])
            _dump("d_key2", key2[:])

            # right-scan keys: missing slots (0.0) flipped to +R_SENT
            msk = sb.tile([P, P], f32)
            nc.vector.tensor_scalar(msk[:], key1[:], 0.0, None, Alu.is_equal)
            r1 = sb.tile([P, P], f32)
            nc.vector.scalar_tensor_tensor(
                r1[:], msk[:], R_SENT, key1[:], Alu.mult, Alu.add
            )
            r2 = sb.tile([P, P], f32)
            nc.vector.scalar_tensor_tensor(
                r2[:], msk[:], R_SENT, key2[:], Alu.mult, Alu.add
            )

            # per-partition scans (free axis); suffix scans via reversed APs
            l1s = sb.tile([P, P], f32)
            nc.vector.tensor_tensor_scan(
                l1s[:], key1[:], key1[:], 0.0, Alu.max, Alu.max
            )
            l2s = sb.tile([P, P], f32)
            nc.vector.tensor_tensor_scan(
                l2s[:], key2[:], key2[:], 0.0, Alu.max, Alu.max
            )
            r1s = sb.tile([P, P], f32)
            nc.vector.tensor_tensor_scan(
                r1s[:, P - 1 :: -1],
                r1[:, P - 1 :: -1],
                r1[:, P - 1 :: -1],
                R_SENT,
                Alu.min,
                Alu.min,
            )
            r2s = sb.tile([P, P], f32)
            nc.vector.tensor_tensor_scan(
                r2s[:, P - 1 :: -1],
                r2[:, P - 1 :: -1],
                r2[:, P - 1 :: -1],
                R_SENT,
                Alu.min,
                Alu.min,
            )

            # cross-partition carry: transpose per-partition totals, exclusive
            # scan along the row, transpose back, combine
            totL = sb.tile([P, 2], f32)
            nc.vector.tensor_copy(totL[:, 0:1], l1s[:, P - 1 : P])
            nc.vector.tensor_copy(totL[:, 1:2], l2s[:, P - 1 : P])
            totR = sb.tile([P, 2], f32)
            nc.vector.tensor_copy(totR[:, 0:1], r1s[:, 0:1])
            nc.vector.tensor_copy(totR[:, 1:2], r2s[:, 0:1])
            totL_tp = ps.tile([2, P], f32)
            nc.tensor.transpose(totL_tp[:], totL[:], ident_t[:])
            totL_T = sb.tile([2, P], f32)
            nc.vector.tensor_copy(totL_T[:], totL_tp[:])
            totR_tp = ps.tile([2, P], f32)
            nc.tensor.transpose(totR_tp[:], totR[:], ident_t[:])
            totR_T = sb.tile([2, P], f32)
            nc.vector.tensor_copy(totR_T[:], totR_tp[:])

            exL = sb.tile([2, P], f32)
            nc.vector.memset(exL[:, 0:1], 0.0)
            nc.vector.tensor_tensor_scan(
                exL[:, 1:P],
                totL_T[:, 0 : P - 1],
                totL_T[:, 0 : P - 1],
                0.0,
                Alu.max,
                Alu.max,
            )
            exR = sb.tile([2, P], f32)
            nc.vector.memset(exR[:, P - 1 : P], R_SENT)
            nc.vector.tensor_tensor_scan(
                exR[:, P - 2 :: -1],
                totR_T[:, P - 1 : 0 : -1],
                totR_T[:, P - 1 : 0 : -1],
                R_SENT,
                Alu.min,
                Alu.min,
            )
            exL_tp = ps.tile([P, 2], f32)
            nc.tensor.transpose(exL_tp[:], exL[:], ident_t[0:2, 0:2])
            carryL = sb.tile([P, 2], f32)
            nc.vector.tensor_copy(carryL[:], exL_tp[:])
            exR_tp = ps.tile([P, 2], f32)
            nc.tensor.transpose(exR_tp[:], exR[:], ident_t[0:2, 0:2])
            carryR = sb.tile([P, 2], f32)
            nc.vector.tensor_copy(carryR[:], exR_tp[:])
            nc.vector.tensor_scalar_max(l1s[:], l1s[:], carryL[:, 0:1])
            nc.vector.tensor_scalar_max(l2s[:], l2s[:], carryL[:, 1:2])
            nc.vector.tensor_scalar_min(r1s[:], r1s[:], carryR[:, 0:1])
            nc.vector.tensor_scalar_min(r2s[:], r2s[:], carryR[:, 1:2])
            _dump("d_l1s", l1s[:])
            _dump("d_l2s", l2s[:])
            _dump("d_r1s", r1s[:])
            _dump("d_r2s", r2s[:])

            # decode: slot = key1>>7, j = ((key1&127)<<6) | (key2&63)
            l1i = sb.tile([P, P], i32)
            nc.vector.tensor_copy(l1i[:], l1s[:])
            l2i = sb.tile([P, P], i32)
            nc.vector.tensor_copy(l2i[:], l2s[:])
            r1i = sb.tile([P, P], i32)
            nc.vector.tensor_copy(r1i[:], r1s[:])
            r2i = sb.tile([P, P], i32)
            nc.vector.tensor_copy(r2i[:], r2s[:])

            slot_l = sb.tile([P, P], i32)
            nc.vector.tensor_scalar(slot_l[:], l1i[:], 7, None, Alu.arith_shift_right)
            slot_r = sb.tile([P, P], i32)
            nc.vector.tensor_scalar(slot_r[:], r1i[:], 7, None, Alu.arith_shift_right)
            jhl = sb.tile([P, P], i32)
            nc.vector.tensor_scalar(
                jhl[:], l1i[:], 127, 6, Alu.bitwise_and, Alu.arith_shift_left
            )
            jll = sb.tile([P, P], i32)
            nc.vector.tensor_scalar(jll[:], l2i[:], 63, None, Alu.bitwise_and)
            jl = sb.tile([P, P], i32)
            nc.vector.tensor_tensor(jl[:], jhl[:], jll[:], Alu.bitwise_or)
            jhr = sb.tile([P, P], i32)
            nc.vector.tensor_scalar(
                jhr[:], r1i[:], 127, 6, Alu.bitwise_and, Alu.arith_shift_left
            )
            jlr = sb.tile([P, P], i32)
            nc.vector.tensor_scalar(jlr[:], r2i[:], 63, None, Alu.bitwise_and)
            jr = sb.tile([P, P], i32)
            nc.vector.tensor_tensor(jr[:], jhr[:], jlr[:], Alu.bitwise_or)

            pos = sb.tile([P, P], i32)
            nc.gpsimd.iota(
                pos[:], pattern=[[1, P]], base=16384, channel_multiplier=P
            )
            dl = sb.tile([P, P], i32)
            nc.vector.tensor_tensor(dl[:], pos[:], slot_l[:], Alu.subtract)
            drr = sb.tile([P, P], i32)
            nc.vector.tensor_tensor(drr[:], slot_r[:], pos[:], Alu.subtract)
            m_l = sb.tile([P, P], i32)
            nc.vector.tensor_tensor(m_l[:], dl[:], drr[:], Alu.is_lt)
            m_r = sb.tile([P, P], i32)
            nc.vector.tensor_tensor(m_r[:], drr[:], dl[:], Alu.is_lt)
            src = sb.tile([P, P], i32)
            nc.vector.tensor_tensor(src[:], jl[:], jr[:], Alu.min)
            nc.vector.copy_predicated(src[:], m_r[:], jr[:])
            nc.vector.copy_predicated(src[:], m_l[:], jl[:])
            _dump("d_src", src[:])
            _dump("d_pos", pos[:])
            srcf = sb.tile([P, P], f32)
            nc.vector.tensor_copy(srcf[:], src[:])

            # extract this core's 16 blocks of 128 indices: G[r, g] =
            # src[16m+g, r] via one-hot matmul, then gather + write out
            g_ps = ps.tile([P, NBLK], f32)
            nc.tensor.matmul(g_ps[:], srcf[:], sel_t[:], start=True, stop=True)
            g_i = sb.tile([P, NBLK], i32)
            nc.vector.tensor_copy(g_i[:], g_ps[:])
            _dump("d_gi", g_i[:])

            for b in range(NBLK):
                gt = gp.tile([P, C], f32, tag="g")
                nc.gpsimd.indirect_dma_start(
                    out=gt[:],
                    out_offset=None,
                    in_=xc[:],
                    in_offset=bass.IndirectOffsetOnAxis(
                        ap=g_i[:, b : b + 1], axis=0
                    ),
                )
                nc.sync.dma_start(y[b * P : (b + 1) * P, :], gt[:])

    return {"y": y}


def host_inputs(x_coarse, keep_idx):
    x_coarse = np.ascontiguousarray(np.asarray(x_coarse), dtype=np.float32)
    ki = np.ascontiguousarray(np.asarray(keep_idx), dtype=np.int32).reshape(-1)
    # token order t = c*128 + p
    keep_w = np.ascontiguousarray(ki.reshape(KC, P).T)
    # wrapped-16 indices, replicated across the 8 gpsimd cores
    k16 = ki[np.arange(E_COARSE // 16)[None, :] * 16 + (np.arange(P) % 16)[:, None]]
    keep16 = np.ascontiguousarray(k16.astype(np.int16))
    pp = np.arange(P)
    cc = np.arange(KC)
    jhi_a = (2 * cc[None, :] + (pp[:, None] >= 64) + 2097152.0).astype(np.float32)
    jlo_a = ((pp[:, None] & 63) + 2097152.0).astype(np.float32)
    ident_a = np.eye(P, dtype=np.float32)
    base = {
        "xc": x_coarse,
        "keep_w": keep_w,
        "keep16": keep16,
        "jhi": np.ascontiguousarray(jhi_a),
        "jlo": np.ascontiguousarray(jlo_a),
        "ident": ident_a,
    }
    in_maps = []
    for m in range(N_CORES):
        sel_a = np.zeros((P, NBLK), dtype=np.float32)
        sel_a[16 * m + np.arange(NBLK), np.arange(NBLK)] = 1.0
        in_maps.append(dict(base, sel=sel_a))
    return in_maps


def _get_nc():
    if "nc" in _NC_CACHE:
        return _NC_CACHE["nc"]
    _ensure_paths()
    from concourse import bass, mybir
    import concourse.bacc as bacc
    import concourse.tile as tile

    nc = bacc.Bacc("TRN2", target_bir_lowering=False, debug=False, dynamic_dma_scratch_size=16384)
    build_program(nc, bass, mybir, tile)
    nc.compile()
    _NC_CACHE["nc"] = nc
    return nc


def run_on_hw(in_maps, trace=False, **kwargs):
    _ensure_paths()
    from concourse.bass_utils import run_bass_kernel_spmd

    nc = _get_nc()
    return run_bass_kernel_spmd(
        nc, in_maps, core_ids=list(range(N_CORES)), trace=trace, **kwargs
    )


def kernel(x_coarse, keep_idx, E_fine=None, **_unused):
    in_maps = host_inputs(x_coarse, keep_idx)
    res = run_on_hw(in_maps)
    out = np.concatenate([res.results[m]["y"] for m in range(N_CORES)], axis=0)
    return np.ascontiguousarray(out.astype(np.float32, copy=False))


# revision 15
# speedup vs baseline: 2.4734x; 1.7135x over previous
"""MeshUnpool Trainium2 kernel.

For every fine edge slot s in [0, 16384):
  - if s is a kept slot (s == keep_idx[j] for some j): out[s] = x_coarse[j]
  - else: out[s] = x_coarse[argmin_j |keep_idx[j] - s|]  (first-min tie-break)

Every output row is a gathered x_coarse row; the device computes the gather
index per slot with an O(E_fine) scan algorithm instead of the naive
(16384 x 8192) distance matrix:

  1. build the slot table with a matmul scatter: one-hot matrices
     A[j, p] = (keep_j >> 7 == p) and C[j, f] = (keep_j & 127 == f) are
     built with two wide compares, then T[p, f] = sum_j A[j,p]*C[j,f]*v_j
     accumulates on the TensorEngine (v = j_hi+1 and j_lo payloads).
     T is the scatter: slot s = 128p+f holds its keep's j, or 0 if missing.
  2. prefix-max scan of key1/key2 over slots -> nearest kept slot <= s with
     its j riding along; suffix-min scan -> nearest kept slot >= s.
     Scans run per-partition with tensor_tensor_scan plus a transposed
     cross-partition carry fixup.
  3. elementwise distance compare + first-min tie-break -> src_idx per slot.
  4. a one-hot matmul extracts this core's 2048 indices, then 16 indirect
     row-gather DMAs (128 rows x 2 KB) pull the output rows from x_coarse.

Work is sharded over 8 cores by rows of the fine-edge dim; x_coarse and
keep_idx are replicated; each core fills its 2048-row slice.
"""

import os
import sys

import numpy as np

E_FINE = 16384
E_COARSE = 8192
C = 512
N_CORES = 8
SLICE = E_FINE // N_CORES  # 2048
P = 128
NBLK = SLICE // P  # 16 gather blocks per core
KC = E_COARSE // P  # 64 keep columns (j = 64*p + c)

KEY_OFF = 2097152.0  # +2^21 added to keys so "missing slot" == 0.0
R_SENT = 8388608.0   # +2^23: flipped sentinel for the suffix-min scans

_NC_CACHE = {}
_DUMP = None  # debug hook: _DUMP(name, ap) dumps an AP to a DRAM tensor


def _dump(name, ap):
    if _DUMP is not None:
        _DUMP(name, ap)


def _ensure_paths():
    for p in ("/opt/trn_rl_repo", "/root/.axon_site/_ro/trn_rl_repo"):
        if os.path.isdir(p) and p not in sys.path:
            sys.path.append(p)


def build_program(nc, bass, mybir, tile):
    f32 = mybir.dt.float32
    i32 = mybir.dt.int32
    Alu = mybir.AluOpType

    i16 = mybir.dt.int16

    bf16 = mybir.dt.bfloat16

    xc = nc.dram_tensor("xc", [E_COARSE, C], f32, kind="ExternalInput")
    # keep_w[jp, c] = keep_idx[c*128 + jp]  (j on partitions per chunk c)
    keep_w = nc.dram_tensor("keep_w", [P, KC], i32, kind="ExternalInput")
    # jhi1[jp, c] = (j >> 6) + 1 and jlo[jp, 0] = j & 63  for j = c*128+jp
    jhi1_in = nc.dram_tensor("jhi1", [P, KC], bf16, kind="ExternalInput")
    jlo_in = nc.dram_tensor("jlo", [P, 1], f32, kind="ExternalInput")
    # iota_b[p, f] = f  (same row on every partition)
    iota_in = nc.dram_tensor("iotab", [P, P], bf16, kind="ExternalInput")
    ident = nc.dram_tensor("ident", [P, P], f32, kind="ExternalInput")
    sel = nc.dram_tensor("sel", [P, NBLK], f32, kind="ExternalInput")
    y = nc.dram_tensor("y", [SLICE, C], f32, kind="ExternalOutput")

    with tile.TileContext(nc) as tc:
        with (
            tc.tile_pool(name="sb", bufs=1) as sb,
            tc.tile_pool(name="ps", bufs=1, space="PSUM") as ps,
            tc.tile_pool(name="gp", bufs=4) as gp,
        ):
            keep_t = sb.tile([P, KC], i32)
            nc.sync.dma_start(keep_t[:], keep_w[:])
            jhi1_t = sb.tile([P, KC], bf16)
            nc.sync.dma_start(jhi1_t[:], jhi1_in[:])
            jlo_t = sb.tile([P, 1], f32)
            nc.sync.dma_start(jlo_t[:], jlo_in[:])
            iota_t = sb.tile([P, P], bf16)
            nc.sync.dma_start(iota_t[:], iota_in[:])
            ident_t = sb.tile([P, P], f32)
            nc.sync.dma_start(ident_t[:], ident[:])
            sel_t = sb.tile([P, NBLK], f32)
            nc.sync.dma_start(sel_t[:], sel[:])

            # slot position iota: pos[p, f] = 16384 + 128p + f (the 16384
            # base makes the +2^21 key offset fall out of 128*pos)
            pos = sb.tile([P, P], i32)
            nc.gpsimd.iota(
                pos[:], pattern=[[1, P]], base=16384, channel_multiplier=P
            )

            # matmul scatter: T[p, f] = sum_j [keep_j>>7 == p][keep_j&127 == f] * v_j
            hi_i = sb.tile([P, KC], i32)
            nc.vector.tensor_scalar(hi_i[:], keep_t[:], 7, None, Alu.arith_shift_right)
            lo_i = sb.tile([P, KC], i32)
            nc.vector.tensor_scalar(lo_i[:], keep_t[:], 127, None, Alu.bitwise_and)
            hi_b = sb.tile([P, KC], bf16)
            nc.vector.tensor_copy(hi_b[:], hi_i[:])
            lo_b = sb.tile([P, KC], bf16)
            nc.vector.tensor_copy(lo_b[:], lo_i[:])

            a_all = sb.tile([P, KC, P], bf16)
            nc.vector.tensor_tensor(
                a_all[:],
                hi_b[:].unsqueeze(2).to_broadcast([P, KC, P]),
                iota_t[:].unsqueeze(1).to_broadcast([P, KC, P]),
                Alu.is_equal,
            )
            cc = sb.tile([P, KC, 2 * P], bf16)
            nc.vector.tensor_tensor(
                cc[:, :, 0:P],
                lo_b[:].unsqueeze(2).to_broadcast([P, KC, P]),
                iota_t[:].unsqueeze(1).to_broadcast([P, KC, P]),
                Alu.is_equal,
            )
            # payload copies: Ch = C * (j_hi + 1), Cl = C * j_lo
            nc.vector.tensor_tensor(
                cc[:, :, P : 2 * P],
                cc[:, :, 0:P],
                jhi1_t[:].unsqueeze(2).to_broadcast([P, KC, P]),
                Alu.mult,
            )
            nc.vector.tensor_scalar_mul(cc[:, :, 0:P], cc[:, :, 0:P], jlo_t[:, 0:1])

            tpsum = ps.tile([P, 2 * P], f32)
            for c in range(KC):
                nc.tensor.matmul(
                    tpsum[:],
                    a_all[:, c, :],
                    cc[:, c, :],
                    start=(c == 0),
                    stop=(c == KC - 1),
                )
            # tpsum[:, P:2P] = T_hi1 (j_hi+1, 0 if missing), [:, 0:P] = T_lo
            posf = sb.tile([P, P], f32)
            nc.vector.tensor_copy(posf[:], pos[:])
            m_kept = sb.tile([P, P], f32)
            nc.vector.tensor_scalar(
                m_kept[:], tpsum[:, P : 2 * P], 0.0, None, Alu.is_gt
            )
            th = sb.tile([P, P], f32)
            nc.vector.tensor_scalar(th[:], tpsum[:, P : 2 * P], 1.0, None, Alu.subtract)
            # key1 = kept * (128*pos + j_hi); 128*pos = 128*slot + 2^21
            k1r = sb.tile([P, P], f32)
            nc.vector.scalar_tensor_tensor(
                k1r[:], posf[:], 128.0, th[:], Alu.mult, Alu.add
            )
            key1 = sb.tile([P, P], f32)
            nc.vector.tensor_tensor(key1[:], k1r[:], m_kept[:], Alu.mult)
            # key2 = kept * (64*pos + j_lo); 64*pos = 64*slot + 2^20
            k2r = sb.tile([P, P], f32)
            nc.vector.scalar_tensor_tensor(
                k2r[:], posf[:], 64.0, tpsum[:, 0:P], Alu.mult, Alu.add
            )
            key2 = sb.tile([P, P], f32)
            nc.vector.tensor_tensor(key2[:], k2r[:], m_kept[:], Alu.mult)
            _dump("d_key1", key1[:])
            _dump("d_key2", key2[:])

            # right-scan keys: missing slots (0.0) flipped to +R_SENT
            msk = sb.tile([P, P], f32)
            nc.vector.tensor_scalar(msk[:], key1[:], 0.0, None, Alu.is_equal)
            r1 = sb.tile([P, P], f32)
            nc.vector.scalar_tensor_tensor(
                r1[:], msk[:], R_SENT, key1[:], Alu.mult, Alu.add
            )
            r2 = sb.tile([P, P], f32)
            nc.vector.scalar_tensor_tensor(
                r2[:], msk[:], R_SENT, key2[:], Alu.mult, Alu.add
            )

            # per-partition scans (free axis); suffix scans via reversed APs
            l1s = sb.tile([P, P], f32)
            nc.vector.tensor_tensor_scan(
                l1s[:], key1[:], key1[:], 0.0, Alu.max, Alu.max
            )
            l2s = sb.tile([P, P], f32)
            nc.vector.tensor_tensor_scan(
                l2s[:], key2[:], key2[:], 0.0, Alu.max, Alu.max
            )
            r1s = sb.tile([P, P], f32)
            nc.vector.tensor_tensor_scan(
                r1s[:, P - 1 :: -1],
                r1[:, P - 1 :: -1],
                r1[:, P - 1 :: -1],
                R_SENT,
                Alu.min,
                Alu.min,
            )
            r2s = sb.tile([P, P], f32)
            nc.vector.tensor_tensor_scan(
                r2s[:, P - 1 :: -1],
                r2[:, P - 1 :: -1],
                r2[:, P - 1 :: -1],
                R_SENT,
                Alu.min,
                Alu.min,
            )

            # cross-partition carry: transpose per-partition totals, exclusive
            # scan along the row, transpose back, combine
            totL = sb.tile([P, 2], f32)
            nc.vector.tensor_copy(totL[:, 0:1], l1s[:, P - 1 : P])
            nc.vector.tensor_copy(totL[:, 1:2], l2s[:, P - 1 : P])
            totR = sb.tile([P, 2], f32)
            nc.vector.tensor_copy(totR[:, 0:1], r1s[:, 0:1])
            nc.vector.tensor_copy(totR[:, 1:2], r2s[:, 0:1])
            totL_tp = ps.tile([2, P], f32)
            nc.tensor.transpose(totL_tp[:], totL[:], ident_t[:])
            totL_T = sb.tile([2, P], f32)
            nc.vector.tensor_copy(totL_T[:], totL_tp[:])
            totR_tp = ps.tile([2, P], f32)
            nc.tensor.transpose(totR_tp[:], totR[:], ident_t[:])
            totR_T = sb.tile([2, P], f32)
            nc.vector.tensor_copy(totR_T[:], totR_tp[:])

            exL = sb.tile([2, P], f32)
            nc.vector.memset(exL[:, 0:1], 0.0)
            nc.vector.tensor_tensor_scan(
                exL[:, 1:P],
                totL_T[:, 0 : P - 1],
                totL_T[:, 0 : P - 1],
                0.0,
                Alu.max,
                Alu.max,
            )
            exR = sb.tile([2, P], f32)
            nc.vector.memset(exR[:, P - 1 : P], R_SENT)
            nc.vector.tensor_tensor_scan(
                exR[:, P - 2 :: -1],
                totR_T[:, P - 1 : 0 : -1],
                totR_T[:, P - 1 : 0 : -1],
                R_SENT,
                Alu.min,
                Alu.min,
            )
            exL_tp = ps.tile([P, 2], f32)
            nc.tensor.transpose(exL_tp[:], exL[:], ident_t[0:2, 0:2])
            carryL = sb.tile([P, 2], f32)
            nc.vector.tensor_copy(carryL[:], exL_tp[:])
            exR_tp = ps.tile([P, 2], f32)
            nc.tensor.transpose(exR_tp[:], exR[:], ident_t[0:2, 0:2])
            carryR = sb.tile([P, 2], f32)
            nc.vector.tensor_copy(carryR[:], exR_tp[:])
            nc.vector.tensor_scalar_max(l1s[:], l1s[:], carryL[:, 0:1])
            nc.vector.tensor_scalar_max(l2s[:], l2s[:], carryL[:, 1:2])
            nc.vector.tensor_scalar_min(r1s[:], r1s[:], carryR[:, 0:1])
            nc.vector.tensor_scalar_min(r2s[:], r2s[:], carryR[:, 1:2])
            _dump("d_l1s", l1s[:])
            _dump("d_l2s", l2s[:])
            _dump("d_r1s", r1s[:])
            _dump("d_r2s", r2s[:])

            # decode: slot = key1>>7, j = ((key1&127)<<6) | (key2&63)
            l1i = sb.tile([P, P], i32)
            nc.vector.tensor_copy(l1i[:], l1s[:])
            l2i = sb.tile([P, P], i32)
            nc.vector.tensor_copy(l2i[:], l2s[:])
            r1i = sb.tile([P, P], i32)
            nc.vector.tensor_copy(r1i[:], r1s[:])
            r2i = sb.tile([P, P], i32)
            nc.vector.tensor_copy(r2i[:], r2s[:])

            slot_l = sb.tile([P, P], i32)
            nc.vector.tensor_scalar(slot_l[:], l1i[:], 7, None, Alu.arith_shift_right)
            slot_r = sb.tile([P, P], i32)
            nc.vector.tensor_scalar(slot_r[:], r1i[:], 7, None, Alu.arith_shift_right)
            jhl = sb.tile([P, P], i32)
            nc.vector.tensor_scalar(
                jhl[:], l1i[:], 127, 6, Alu.bitwise_and, Alu.arith_shift_left
            )
            jll = sb.tile([P, P], i32)
            nc.vector.tensor_scalar(jll[:], l2i[:], 63, None, Alu.bitwise_and)
            jl = sb.tile([P, P], i32)
            nc.vector.tensor_tensor(jl[:], jhl[:], jll[:], Alu.bitwise_or)
            jhr = sb.tile([P, P], i32)
            nc.vector.tensor_scalar(
                jhr[:], r1i[:], 127, 6, Alu.bitwise_and, Alu.arith_shift_left
            )
            jlr = sb.tile([P, P], i32)
            nc.vector.tensor_scalar(jlr[:], r2i[:], 63, None, Alu.bitwise_and)
            jr = sb.tile([P, P], i32)
            nc.vector.tensor_tensor(jr[:], jhr[:], jlr[:], Alu.bitwise_or)

            dl = sb.tile([P, P], i32)
            nc.vector.tensor_tensor(dl[:], pos[:], slot_l[:], Alu.subtract)
            drr = sb.tile([P, P], i32)
            nc.vector.tensor_tensor(drr[:], slot_r[:], pos[:], Alu.subtract)
            m_l = sb.tile([P, P], i32)
            nc.vector.tensor_tensor(m_l[:], dl[:], drr[:], Alu.is_lt)
            m_r = sb.tile([P, P], i32)
            nc.vector.tensor_tensor(m_r[:], drr[:], dl[:], Alu.is_lt)
            src = sb.tile([P, P], i32)
            nc.vector.tensor_tensor(src[:], jl[:], jr[:], Alu.min)
            nc.vector.copy_predicated(src[:], m_r[:], jr[:])
            nc.vector.copy_predicated(src[:], m_l[:], jl[:])
            _dump("d_src", src[:])
            _dump("d_pos", pos[:])
            srcf = sb.tile([P, P], f32)
            nc.vector.tensor_copy(srcf[:], src[:])

            # extract this core's 16 blocks of 128 indices: G[r, g] =
            # src[16m+g, r] via one-hot matmul, then gather + write out
            g_ps = ps.tile([P, NBLK], f32)
            nc.tensor.matmul(g_ps[:], srcf[:], sel_t[:], start=True, stop=True)
            g_i = sb.tile([P, NBLK], i32)
            nc.vector.tensor_copy(g_i[:], g_ps[:])
            _dump("d_gi", g_i[:])

            for b in range(NBLK):
                gt = gp.tile([P, C], f32, tag="g")
                nc.gpsimd.indirect_dma_start(
                    out=gt[:],
                    out_offset=None,
                    in_=xc[:],
                    in_offset=bass.IndirectOffsetOnAxis(
                        ap=g_i[:, b : b + 1], axis=0
                    ),
                )
                nc.sync.dma_start(y[b * P : (b + 1) * P, :], gt[:])

    return {"y": y}


def host_inputs(x_coarse, keep_idx):
    import ml_dtypes

    bf = ml_dtypes.bfloat16
    x_coarse = np.ascontiguousarray(np.asarray(x_coarse), dtype=np.float32)
    ki = np.ascontiguousarray(np.asarray(keep_idx), dtype=np.int32).reshape(-1)
    # j = c*128 + jp: keep_w[jp, c] = keep_idx[j]
    keep_w = np.ascontiguousarray(ki.reshape(KC, P).T)
    pp = np.arange(P)
    cc = np.arange(KC)
    jhi1_a = (2 * cc[None, :] + (pp[:, None] >= 64) + 1).astype(bf)
    jlo_a = (pp[:, None] & 63).astype(np.float32)
    iota_a = np.tile(np.arange(P), (P, 1)).astype(bf)
    ident_a = np.eye(P, dtype=np.float32)
    base = {
        "xc": x_coarse,
        "keep_w": keep_w,
        "jhi1": np.ascontiguousarray(jhi1_a),
        "jlo": np.ascontiguousarray(jlo_a),
        "iotab": np.ascontiguousarray(iota_a),
        "ident": ident_a,
    }
    in_maps = []
    for m in range(N_CORES):
        sel_a = np.zeros((P, NBLK), dtype=np.float32)
        sel_a[16 * m + np.arange(NBLK), np.arange(NBLK)] = 1.0
        in_maps.append(dict(base, sel=sel_a))
    return in_maps


def _get_nc():
    if "nc" in _NC_CACHE:
        return _NC_CACHE["nc"]
    _ensure_paths()
    from concourse import bass, mybir
    import concourse.bacc as bacc
    import concourse.tile as tile

    nc = bacc.Bacc("TRN2", target_bir_lowering=False, debug=False, dynamic_dma_scratch_size=16384)
    build_program(nc, bass, mybir, tile)
    nc.compile()
    _NC_CACHE["nc"] = nc
    return nc


def run_on_hw(in_maps, trace=False, **kwargs):
    _ensure_paths()
    from concourse.bass_utils import run_bass_kernel_spmd

    nc = _get_nc()
    return run_bass_kernel_spmd(
        nc, in_maps, core_ids=list(range(N_CORES)), trace=trace, **kwargs
    )


def kernel(x_coarse, keep_idx, E_fine=None, **_unused):
    in_maps = host_inputs(x_coarse, keep_idx)
    res = run_on_hw(in_maps)
    out = np.concatenate([res.results[m]["y"] for m in range(N_CORES)], axis=0)
    return np.ascontiguousarray(out.astype(np.float32, copy=False))


# revision 17
# speedup vs baseline: 2.9744x; 1.2025x over previous
"""MeshUnpool Trainium2 kernel.

For every fine edge slot s in [0, 16384):
  - if s is a kept slot (s == keep_idx[j] for some j): out[s] = x_coarse[j]
  - else: out[s] = x_coarse[argmin_j |keep_idx[j] - s|]  (first-min tie-break)

Every output row is a gathered x_coarse row; the device computes the gather
index per slot with an O(E_fine) scan algorithm instead of the naive
(16384 x 8192) distance matrix:

  1. build the slot table with a matmul scatter: one-hot matrices
     A[j, p] = (keep_j >> 7 == p) and C[j, f] = (keep_j & 127 == f) are
     built with two wide compares, then T[p, f] = sum_j A[j,p]*C[j,f]*v_j
     accumulates on the TensorEngine (v = j_hi+1 and j_lo payloads).
     T is the scatter: slot s = 128p+f holds its keep's j, or 0 if missing.
  2. prefix-max scan of key1/key2 over slots -> nearest kept slot <= s with
     its j riding along; suffix-min scan -> nearest kept slot >= s.
     Scans run per-partition with tensor_tensor_scan plus a transposed
     cross-partition carry fixup.
  3. elementwise distance compare + first-min tie-break -> src_idx per slot.
  4. a one-hot matmul extracts this core's 2048 indices, then 16 indirect
     row-gather DMAs (128 rows x 2 KB) pull the output rows from x_coarse.

Work is sharded over 8 cores by rows of the fine-edge dim; x_coarse and
keep_idx are replicated; each core fills its 2048-row slice.
"""

import os
import sys

import numpy as np

E_FINE = 16384
E_COARSE = 8192
C = 512
N_CORES = 8
SLICE = E_FINE // N_CORES  # 2048
P = 128
NBLK = SLICE // P  # 16 gather blocks per core
KC = E_COARSE // P  # 64 keep columns (j = 64*p + c)

KEY_OFF = 2097152.0  # +2^21 added to keys so "missing slot" == 0.0
R_SENT = 8388608.0   # +2^23: flipped sentinel for the suffix-min scans

_NC_CACHE = {}
_DUMP = None  # debug hook: _DUMP(name, ap) dumps an AP to a DRAM tensor


def _dump(name, ap):
    if _DUMP is not None:
        _DUMP(name, ap)


def _ensure_paths():
    for p in ("/opt/trn_rl_repo", "/root/.axon_site/_ro/trn_rl_repo"):
        if os.path.isdir(p) and p not in sys.path:
            sys.path.append(p)


def build_program(nc, bass, mybir, tile):
    f32 = mybir.dt.float32
    i32 = mybir.dt.int32
    Alu = mybir.AluOpType

    i16 = mybir.dt.int16

    bf16 = mybir.dt.bfloat16

    xc = nc.dram_tensor("xc", [E_COARSE, C], f32, kind="ExternalInput")
    # keep_w[jp, c] = keep_idx[c*128 + jp]  (j on partitions per chunk c)
    keep_w = nc.dram_tensor("keep_w", [P, KC], i32, kind="ExternalInput")
    # jhi1[jp, c] = (j >> 6) + 1 and jlo[jp, 0] = j & 63  for j = c*128+jp
    jhi1_in = nc.dram_tensor("jhi1", [P, KC], bf16, kind="ExternalInput")
    jlo_in = nc.dram_tensor("jlo", [P, 1], f32, kind="ExternalInput")
    # iota_b[p, f] = f  (same row on every partition)
    iota_in = nc.dram_tensor("iotab", [P, P], bf16, kind="ExternalInput")
    ident = nc.dram_tensor("ident", [P, P], f32, kind="ExternalInput")
    sel = nc.dram_tensor("sel", [P, NBLK], f32, kind="ExternalInput")
    y = nc.dram_tensor("y", [SLICE, C], f32, kind="ExternalOutput")

    with tile.TileContext(nc) as tc:
        with (
            tc.tile_pool(name="sb", bufs=1) as sb,
            tc.tile_pool(name="ps", bufs=1, space="PSUM") as ps,
            tc.tile_pool(name="gp", bufs=8) as gp,
        ):
            keep_t = sb.tile([P, KC], i32)
            nc.sync.dma_start(keep_t[:], keep_w[:])
            jhi1_t = sb.tile([P, KC], bf16)
            nc.sync.dma_start(jhi1_t[:], jhi1_in[:])
            jlo_t = sb.tile([P, 1], f32)
            nc.sync.dma_start(jlo_t[:], jlo_in[:])
            iota_t = sb.tile([P, P], bf16)
            nc.sync.dma_start(iota_t[:], iota_in[:])
            ident_t = sb.tile([P, P], f32)
            nc.sync.dma_start(ident_t[:], ident[:])
            sel_t = sb.tile([P, NBLK], f32)
            nc.sync.dma_start(sel_t[:], sel[:])

            # slot position iota: pos[p, f] = 16384 + 128p + f (the 16384
            # base makes the +2^21 key offset fall out of 128*pos)
            pos = sb.tile([P, P], i32)
            nc.gpsimd.iota(
                pos[:], pattern=[[1, P]], base=16384, channel_multiplier=P
            )

            # matmul scatter: T[p, f] = sum_j [keep_j>>7 == p][keep_j&127 == f] * v_j
            hi_i = sb.tile([P, KC], i32)
            nc.vector.tensor_scalar(hi_i[:], keep_t[:], 7, None, Alu.arith_shift_right)
            lo_i = sb.tile([P, KC], i32)
            nc.vector.tensor_scalar(lo_i[:], keep_t[:], 127, None, Alu.bitwise_and)
            hi_b = sb.tile([P, KC], bf16)
            nc.vector.tensor_copy(hi_b[:], hi_i[:])
            lo_b = sb.tile([P, KC], bf16)
            nc.vector.tensor_copy(lo_b[:], lo_i[:])

            # split builds into halves so the PE can start on half 0 while
            # the vector engine builds half 1; A on gpsimd runs in parallel
            # with C/Ch on vector, Cl on the scalar engine
            a_all = sb.tile([P, KC, P], bf16)
            cmat = sb.tile([P, KC, P], bf16)
            chmat = sb.tile([P, KC, P], bf16)
            clmat = sb.tile([P, KC, P], bf16)
            HC = KC // 2
            for h in range(2):
                cs = slice(h * HC, (h + 1) * HC)
                nc.vector.tensor_tensor(
                    a_all[:, cs, :],
                    hi_b[:, cs].unsqueeze(2).to_broadcast([P, HC, P]),
                    iota_t[:].unsqueeze(1).to_broadcast([P, HC, P]),
                    Alu.is_equal,
                )
                nc.vector.tensor_tensor(
                    cmat[:, cs, :],
                    lo_b[:, cs].unsqueeze(2).to_broadcast([P, HC, P]),
                    iota_t[:].unsqueeze(1).to_broadcast([P, HC, P]),
                    Alu.is_equal,
                )
                nc.vector.tensor_tensor(
                    chmat[:, cs, :],
                    cmat[:, cs, :],
                    jhi1_t[:, cs].unsqueeze(2).to_broadcast([P, HC, P]),
                    Alu.mult,
                )
                nc.scalar.mul(clmat[:, cs, :], cmat[:, cs, :], jlo_t[:, 0:1])

            tph = ps.tile([P, P], f32)
            tpl = ps.tile([P, P], f32)
            for c in range(KC):
                nc.tensor.matmul(
                    tph[:],
                    a_all[:, c, :],
                    chmat[:, c, :],
                    start=(c == 0),
                    stop=(c == KC - 1),
                )
                nc.tensor.matmul(
                    tpl[:],
                    a_all[:, c, :],
                    clmat[:, c, :],
                    start=(c == 0),
                    stop=(c == KC - 1),
                )
            posf = sb.tile([P, P], f32)
            nc.vector.tensor_copy(posf[:], pos[:])
            m_kept = sb.tile([P, P], f32)
            nc.vector.tensor_scalar(m_kept[:], tph[:], 0.0, None, Alu.is_gt)
            th = sb.tile([P, P], f32)
            nc.vector.tensor_scalar(th[:], tph[:], 1.0, None, Alu.subtract)
            # key1 = kept * (128*pos + j_hi); 128*pos = 128*slot + 2^21
            k1r = sb.tile([P, P], f32)
            nc.vector.scalar_tensor_tensor(
                k1r[:], posf[:], 128.0, th[:], Alu.mult, Alu.add
            )
            key1 = sb.tile([P, P], f32)
            nc.vector.tensor_tensor(key1[:], k1r[:], m_kept[:], Alu.mult)
            # key2 = kept * (64*pos + j_lo); 64*pos = 64*slot + 2^20
            k2r = sb.tile([P, P], f32)
            nc.vector.scalar_tensor_tensor(
                k2r[:], posf[:], 64.0, tpl[:], Alu.mult, Alu.add
            )
            key2 = sb.tile([P, P], f32)
            nc.vector.tensor_tensor(key2[:], k2r[:], m_kept[:], Alu.mult)
            _dump("d_key1", key1[:])
            _dump("d_key2", key2[:])

            # right-scan keys: missing slots (0.0) flipped to +R_SENT
            msk = sb.tile([P, P], f32)
            nc.vector.tensor_scalar(msk[:], key1[:], 0.0, None, Alu.is_equal)
            r1 = sb.tile([P, P], f32)
            nc.vector.scalar_tensor_tensor(
                r1[:], msk[:], R_SENT, key1[:], Alu.mult, Alu.add
            )
            r2 = sb.tile([P, P], f32)
            nc.vector.scalar_tensor_tensor(
                r2[:], msk[:], R_SENT, key2[:], Alu.mult, Alu.add
            )

            # per-partition scans (free axis); suffix scans via reversed APs
            l1s = sb.tile([P, P], f32)
            nc.vector.tensor_tensor_scan(
                l1s[:], key1[:], key1[:], 0.0, Alu.max, Alu.max
            )
            l2s = sb.tile([P, P], f32)
            nc.vector.tensor_tensor_scan(
                l2s[:], key2[:], key2[:], 0.0, Alu.max, Alu.max
            )
            r1s = sb.tile([P, P], f32)
            nc.vector.tensor_tensor_scan(
                r1s[:, P - 1 :: -1],
                r1[:, P - 1 :: -1],
                r1[:, P - 1 :: -1],
                R_SENT,
                Alu.min,
                Alu.min,
            )
            r2s = sb.tile([P, P], f32)
            nc.vector.tensor_tensor_scan(
                r2s[:, P - 1 :: -1],
                r2[:, P - 1 :: -1],
                r2[:, P - 1 :: -1],
                R_SENT,
                Alu.min,
                Alu.min,
            )

            # cross-partition carry: transpose per-partition totals, exclusive
            # scan along the row, transpose back, combine
            totL = sb.tile([P, 2], f32)
            nc.vector.tensor_copy(totL[:, 0:1], l1s[:, P - 1 : P])
            nc.vector.tensor_copy(totL[:, 1:2], l2s[:, P - 1 : P])
            totR = sb.tile([P, 2], f32)
            nc.vector.tensor_copy(totR[:, 0:1], r1s[:, 0:1])
            nc.vector.tensor_copy(totR[:, 1:2], r2s[:, 0:1])
            totL_tp = ps.tile([2, P], f32)
            nc.tensor.transpose(totL_tp[:], totL[:], ident_t[:])
            totL_T = sb.tile([2, P], f32)
            nc.vector.tensor_copy(totL_T[:], totL_tp[:])
            totR_tp = ps.tile([2, P], f32)
            nc.tensor.transpose(totR_tp[:], totR[:], ident_t[:])
            totR_T = sb.tile([2, P], f32)
            nc.vector.tensor_copy(totR_T[:], totR_tp[:])

            exL = sb.tile([2, P], f32)
            nc.vector.memset(exL[:, 0:1], 0.0)
            nc.vector.tensor_tensor_scan(
                exL[:, 1:P],
                totL_T[:, 0 : P - 1],
                totL_T[:, 0 : P - 1],
                0.0,
                Alu.max,
                Alu.max,
            )
            exR = sb.tile([2, P], f32)
            nc.vector.memset(exR[:, P - 1 : P], R_SENT)
            nc.vector.tensor_tensor_scan(
                exR[:, P - 2 :: -1],
                totR_T[:, P - 1 : 0 : -1],
                totR_T[:, P - 1 : 0 : -1],
                R_SENT,
                Alu.min,
                Alu.min,
            )
            exL_tp = ps.tile([P, 2], f32)
            nc.tensor.transpose(exL_tp[:], exL[:], ident_t[0:2, 0:2])
            carryL = sb.tile([P, 2], f32)
            nc.vector.tensor_copy(carryL[:], exL_tp[:])
            exR_tp = ps.tile([P, 2], f32)
            nc.tensor.transpose(exR_tp[:], exR[:], ident_t[0:2, 0:2])
            carryR = sb.tile([P, 2], f32)
            nc.vector.tensor_copy(carryR[:], exR_tp[:])
            nc.vector.tensor_scalar_max(l1s[:], l1s[:], carryL[:, 0:1])
            nc.vector.tensor_scalar_max(l2s[:], l2s[:], carryL[:, 1:2])
            nc.vector.tensor_scalar_min(r1s[:], r1s[:], carryR[:, 0:1])
            nc.vector.tensor_scalar_min(r2s[:], r2s[:], carryR[:, 1:2])
            _dump("d_l1s", l1s[:])
            _dump("d_l2s", l2s[:])
            _dump("d_r1s", r1s[:])
            _dump("d_r2s", r2s[:])

            # decode: slot = key1>>7, j = ((key1&127)<<6) | (key2&63)
            l1i = sb.tile([P, P], i32)
            nc.vector.tensor_copy(l1i[:], l1s[:])
            l2i = sb.tile([P, P], i32)
            nc.vector.tensor_copy(l2i[:], l2s[:])
            r1i = sb.tile([P, P], i32)
            nc.vector.tensor_copy(r1i[:], r1s[:])
            r2i = sb.tile([P, P], i32)
            nc.vector.tensor_copy(r2i[:], r2s[:])

            slot_l = sb.tile([P, P], i32)
            nc.vector.tensor_scalar(slot_l[:], l1i[:], 7, None, Alu.arith_shift_right)
            slot_r = sb.tile([P, P], i32)
            nc.vector.tensor_scalar(slot_r[:], r1i[:], 7, None, Alu.arith_shift_right)
            jhl = sb.tile([P, P], i32)
            nc.vector.tensor_scalar(
                jhl[:], l1i[:], 127, 6, Alu.bitwise_and, Alu.arith_shift_left
            )
            jll = sb.tile([P, P], i32)
            nc.vector.tensor_scalar(jll[:], l2i[:], 63, None, Alu.bitwise_and)
            jl = sb.tile([P, P], i32)
            nc.vector.tensor_tensor(jl[:], jhl[:], jll[:], Alu.bitwise_or)
            jhr = sb.tile([P, P], i32)
            nc.vector.tensor_scalar(
                jhr[:], r1i[:], 127, 6, Alu.bitwise_and, Alu.arith_shift_left
            )
            jlr = sb.tile([P, P], i32)
            nc.vector.tensor_scalar(jlr[:], r2i[:], 63, None, Alu.bitwise_and)
            jr = sb.tile([P, P], i32)
            nc.vector.tensor_tensor(jr[:], jhr[:], jlr[:], Alu.bitwise_or)

            dl = sb.tile([P, P], i32)
            nc.vector.tensor_tensor(dl[:], pos[:], slot_l[:], Alu.subtract)
            drr = sb.tile([P, P], i32)
            nc.vector.tensor_tensor(drr[:], slot_r[:], pos[:], Alu.subtract)
            m_l = sb.tile([P, P], i32)
            nc.vector.tensor_tensor(m_l[:], dl[:], drr[:], Alu.is_lt)
            m_r = sb.tile([P, P], i32)
            nc.vector.tensor_tensor(m_r[:], drr[:], dl[:], Alu.is_lt)
            src = sb.tile([P, P], i32)
            nc.vector.tensor_tensor(src[:], jl[:], jr[:], Alu.min)
            nc.vector.copy_predicated(src[:], m_r[:], jr[:])
            nc.vector.copy_predicated(src[:], m_l[:], jl[:])
            _dump("d_src", src[:])
            _dump("d_pos", pos[:])
            srcf = sb.tile([P, P], f32)
            nc.vector.tensor_copy(srcf[:], src[:])

            # extract this core's 16 blocks of 128 indices: G[r, g] =
            # src[16m+g, r] via one-hot matmul, then gather + write out
            g_ps = ps.tile([P, NBLK], f32)
            nc.tensor.matmul(g_ps[:], srcf[:], sel_t[:], start=True, stop=True)
            g_i = sb.tile([P, NBLK], i32)
            nc.vector.tensor_copy(g_i[:], g_ps[:])
            _dump("d_gi", g_i[:])

            for b in range(NBLK):
                gt = gp.tile([P, C], f32, tag="g")
                nc.gpsimd.indirect_dma_start(
                    out=gt[:],
                    out_offset=None,
                    in_=xc[:],
                    in_offset=bass.IndirectOffsetOnAxis(
                        ap=g_i[:, b : b + 1], axis=0
                    ),
                )
                nc.sync.dma_start(y[b * P : (b + 1) * P, :], gt[:])

    return {"y": y}


def host_inputs(x_coarse, keep_idx):
    import ml_dtypes

    bf = ml_dtypes.bfloat16
    x_coarse = np.ascontiguousarray(np.asarray(x_coarse), dtype=np.float32)
    ki = np.ascontiguousarray(np.asarray(keep_idx), dtype=np.int32).reshape(-1)
    # j = c*128 + jp: keep_w[jp, c] = keep_idx[j]
    keep_w = np.ascontiguousarray(ki.reshape(KC, P).T)
    pp = np.arange(P)
    cc = np.arange(KC)
    jhi1_a = (2 * cc[None, :] + (pp[:, None] >= 64) + 1).astype(bf)
    jlo_a = (pp[:, None] & 63).astype(np.float32)
    iota_a = np.tile(np.arange(P), (P, 1)).astype(bf)
    ident_a = np.eye(P, dtype=np.float32)
    base = {
        "xc": x_coarse,
        "keep_w": keep_w,
        "jhi1": np.ascontiguousarray(jhi1_a),
        "jlo": np.ascontiguousarray(jlo_a),
        "iotab": np.ascontiguousarray(iota_a),
        "ident": ident_a,
    }
    in_maps = []
    for m in range(N_CORES):
        sel_a = np.zeros((P, NBLK), dtype=np.float32)
        sel_a[16 * m + np.arange(NBLK), np.arange(NBLK)] = 1.0
        in_maps.append(dict(base, sel=sel_a))
    return in_maps


def _get_nc():
    if "nc" in _NC_CACHE:
        return _NC_CACHE["nc"]
    _ensure_paths()
    from concourse import bass, mybir
    import concourse.bacc as bacc
    import concourse.tile as tile

    nc = bacc.Bacc("TRN2", target_bir_lowering=False, debug=False, dynamic_dma_scratch_size=16384)
    build_program(nc, bass, mybir, tile)
    nc.compile()
    _NC_CACHE["nc"] = nc
    return nc


def run_on_hw(in_maps, trace=False, **kwargs):
    _ensure_paths()
    from concourse.bass_utils import run_bass_kernel_spmd

    nc = _get_nc()
    return run_bass_kernel_spmd(
        nc, in_maps, core_ids=list(range(N_CORES)), trace=trace, **kwargs
    )


def kernel(x_coarse, keep_idx, E_fine=None, **_unused):
    in_maps = host_inputs(x_coarse, keep_idx)
    res = run_on_hw(in_maps)
    out = np.concatenate([res.results[m]["y"] for m in range(N_CORES)], axis=0)
    return np.ascontiguousarray(out.astype(np.float32, copy=False))
